# revision 8
# baseline (speedup 1.0000x reference)
"""Additive (Bahdanau) attention on Trainium2, data-parallel over batch on 8 NeuronCores.

Per core (one batch element):
  qT[h,q] = (queries @ W_q).T, kT[h,k] = (keys @ W_k).T        (PE, fp32; stored bf16)
  pre[h, (k,q)] = qT[h,q] + kT[h,k]                            (DVE tensor_scalar, bf16 4x)
  feat = tanh(pre)                                             (ACT, few big instructions)
  scores[q,k] = sum_h w_v[h]*feat[h,k,q]                       (PE: feat stationary, w_v moving,
                                                                column-accumulated in PSUM)
  scores += maskbias (-1e6 where k >= valid_len)               (DVE)
  e = exp(scores), sums = row-sums via ACT accum_out           (ACT)
  out[q,:] = (e.T @ values) / sums                             (PE transpose + PE matmul + DVE)
"""

import numpy as np

import concourse.bass as bass
import concourse.mybir as mybir
import concourse.tile as tile
from concourse import bacc
from concourse.bass_utils import run_bass_kernel_spmd

B, Q, K, H, D, DV = 8, 256, 256, 256, 256, 256
N_CORES = 8
F32 = mybir.dt.float32
BF16 = mybir.dt.bfloat16
AF = mybir.ActivationFunctionType
KC = 64            # keys per chunk (one big ACT instruction covers KC*Q elements)
NCHUNK = K // KC


def build_nc():
    nc = bacc.Bacc("TRN2", target_bir_lowering=False)
    d_q = nc.dram_tensor("queries", [Q, D], F32, kind="ExternalInput")
    d_k = nc.dram_tensor("keys", [K, D], F32, kind="ExternalInput")
    d_v = nc.dram_tensor("values", [K, DV], F32, kind="ExternalInput")
    d_wq = nc.dram_tensor("W_q", [D, H], F32, kind="ExternalInput")
    d_wk = nc.dram_tensor("W_k", [D, H], F32, kind="ExternalInput")
    d_wv = nc.dram_tensor("w_v", [H, 1], F32, kind="ExternalInput")
    d_mask = nc.dram_tensor("maskbias", [128, K], F32, kind="ExternalInput")
    d_id = nc.dram_tensor("identity", [128, 128], F32, kind="ExternalInput")
    d_out = nc.dram_tensor("out", [Q, DV], F32, kind="ExternalOutput")

    with tile.TileContext(nc) as tc:
        with (
            tc.tile_pool(name="sb", bufs=1) as sb,
            tc.tile_pool(name="feat", bufs=4) as feat_pool,
            tc.tile_pool(name="ps_scores", bufs=1, space=bass.MemorySpace.PSUM) as ps_s,
        ):
            # ------- persistent tiles -------
            ident = sb.tile([128, 128], F32, tag="ident")
            nc.sync.dma_start(ident[:], d_id[:])
            mask_sb = sb.tile([128, K], F32, tag="mask")
            nc.sync.dma_start(mask_sb[:], d_mask[:])
            wv_f = [sb.tile([128, 1], F32, tag=f"wvf{t}", name=f"wvf{t}") for t in range(2)]
            wv_b = [sb.tile([128, 1], BF16, tag=f"wvb{t}", name=f"wvb{t}") for t in range(2)]
            for t in range(2):
                nc.sync.dma_start(wv_f[t][:], d_wv[t * 128:(t + 1) * 128, :])
                nc.vector.tensor_copy(wv_b[t][:], wv_f[t][:])
            vals = [sb.tile([128, DV], F32, tag=f"vals{t}", name=f"vals{t}") for t in range(2)]
            for t in range(2):
                nc.sync.dma_start(vals[t][:], d_v[t * 128:(t + 1) * 128, :])
            qT = [sb.tile([128, Q], BF16, tag=f"qT{t}", name=f"qT{t}") for t in range(2)]
            kT = [sb.tile([128, K], BF16, tag=f"kT{t}", name=f"kT{t}") for t in range(2)]
            # kt2: each kT value duplicated into a bf16 pair -> enables TT 2x_1P packing
            kt2 = [sb.tile([128, 2 * K], BF16, tag=f"kt2{t}", name=f"kt2{t}") for t in range(2)]
            s_ps = [ps_s.tile([128, K], F32, tag=f"s{qt}", name=f"s{qt}") for qt in range(2)]

            # ------- prep: load, transpose, project -------
            with (
                tc.tile_pool(name="prep_sb", bufs=1) as prep_sb,
                tc.tile_pool(name="ps_prep", bufs=2, space=bass.MemorySpace.PSUM) as ps_p,
            ):
                qraw = [prep_sb.tile([128, D], F32, tag=f"qraw{i}", name=f"qraw{i}") for i in range(2)]
                kraw = [prep_sb.tile([128, D], F32, tag=f"kraw{i}", name=f"kraw{i}") for i in range(2)]
                qTT = [prep_sb.tile([128, Q], F32, tag=f"qTT{j}", name=f"qTT{j}") for j in range(2)]
                kTT = [prep_sb.tile([128, K], F32, tag=f"kTT{j}", name=f"kTT{j}") for j in range(2)]
                wq_sb = [prep_sb.tile([128, H], F32, tag=f"wq{j}", name=f"wq{j}") for j in range(2)]
                wk_sb = [prep_sb.tile([128, H], F32, tag=f"wk{j}", name=f"wk{j}") for j in range(2)]
                for i in range(2):
                    nc.sync.dma_start(qraw[i][:], d_q[i * 128:(i + 1) * 128, :])
                    nc.sync.dma_start(kraw[i][:], d_k[i * 128:(i + 1) * 128, :])
                    nc.sync.dma_start(wq_sb[i][:], d_wq[i * 128:(i + 1) * 128, :])
                    nc.sync.dma_start(wk_sb[i][:], d_wk[i * 128:(i + 1) * 128, :])
                # transpose [x, d] -> [d, x] in 128x128 blocks via PE
                for raw, TT in ((qraw, qTT), (kraw, kTT)):
                    for i in range(2):
                        for j in range(2):
                            tp = ps_p.tile([128, 128], F32, tag="tp")
                            nc.tensor.transpose(tp[:], raw[i][:, j * 128:(j + 1) * 128], ident[:])
                            nc.vector.tensor_copy(TT[j][:, i * 128:(i + 1) * 128], tp[:])
                # projections (contract d): xT[t][h,x] = sum_d W[d, t*128+h] * xTT[d, x]
                for w_sb, TT, xT in ((wq_sb, qTT, qT), (wk_sb, kTT, kT)):
                    for t in range(2):
                        pj = ps_p.tile([128, 256], F32, tag="proj")
                        for j in range(2):
                            nc.tensor.matmul(pj[:], w_sb[j][:, t * 128:(t + 1) * 128], TT[j][:],
                                             start=(j == 0), stop=(j == 1))
                        nc.vector.tensor_copy(xT[t][:], pj[:])
                for t in range(2):
                    nc.vector.tensor_copy(
                        kt2[t][:].rearrange("p (k e) -> p k e", e=2),
                        kT[t][:].unsqueeze(2).broadcast_to((128, K, 2)))

            # ------- main loop: tanh features + w_v reduction -------
            for c in range(NCHUNK):
                k0 = c * KC
                feats = []
                for t in range(2):
                    feat = feat_pool.tile([128, KC * Q], BF16, tag="feat")
                    # pre[h, j, qp, e] = qT[h, 2qp+e] + kT[h, k0+j]; bf16 pair APs keep 2x mode
                    in0 = qT[t][:].rearrange("p (qp e) -> p qp e", e=2)
                    in0 = in0.unsqueeze(1).broadcast_to((128, KC, Q // 2, 2))
                    in1 = kt2[t][:, 2 * k0:2 * (k0 + KC)].rearrange("p (k e) -> p k e", e=2)
                    in1 = in1.unsqueeze(2).broadcast_to((128, KC, Q // 2, 2))
                    out = feat[:].rearrange("p (a b c) -> p a b c", a=KC, b=Q // 2)
                    nc.vector.tensor_add(out, in0, in1)
                    nc.scalar.activation(feat[:], feat[:], AF.Tanh)
                    feats.append(feat)
                for j in range(KC):
                    k = k0 + j
                    for qt in range(2):
                        for t in range(2):
                            nc.tensor.matmul(
                                s_ps[qt][:, k:k + 1],
                                feats[t][:, j * Q + qt * 128: j * Q + qt * 128 + 128],
                                wv_b[t][:],
                                start=(t == 0), stop=(t == 1))

            # ------- masked softmax + attention @ values -------
            exp_sb = [sb.tile([128, K], F32, tag=f"exp{qt}", name=f"exp{qt}") for qt in range(2)]
            expT = [sb.tile([128, Q], F32, tag=f"expT{kt}", name=f"expT{kt}") for kt in range(2)]
            sums = [sb.tile([128, 1], F32, tag=f"sum{qt}", name=f"sum{qt}") for qt in range(2)]
            recip = [sb.tile([128, 1], F32, tag=f"rcp{qt}", name=f"rcp{qt}") for qt in range(2)]
            out_sb = [sb.tile([128, DV], F32, tag=f"out{qt}", name=f"out{qt}") for qt in range(2)]
            with tc.tile_pool(name="ps_tail", bufs=2, space=bass.MemorySpace.PSUM) as ps_t:
                for qt in range(2):
                    nc.vector.tensor_add(s_ps[qt][:], s_ps[qt][:], mask_sb[:])
                    nc.scalar.activation(exp_sb[qt][:], s_ps[qt][:], AF.Exp,
                                         accum_out=sums[qt][:])
                for qt in range(2):
                    for kt in range(2):
                        tx = ps_t.tile([128, 128], F32, tag="tx")
                        nc.tensor.transpose(tx[:], exp_sb[qt][:, kt * 128:(kt + 1) * 128], ident[:])
                        nc.vector.tensor_copy(expT[kt][:, qt * 128:(qt + 1) * 128], tx[:])
                for qt in range(2):
                    av = ps_t.tile([128, DV], F32, tag="av")
                    for kt in range(2):
                        nc.tensor.matmul(av[:], expT[kt][:, qt * 128:(qt + 1) * 128], vals[kt][:],
                                         start=(kt == 0), stop=(kt == 1))
                    nc.vector.reciprocal(recip[qt][:], sums[qt][:])
                    nc.vector.tensor_scalar_mul(out_sb[qt][:], av[:], recip[qt][:])
                    nc.sync.dma_start(d_out[qt * 128:(qt + 1) * 128, :], out_sb[qt][:])
    nc.compile()
    return nc


_NC = None


def _get_nc():
    global _NC
    if _NC is None:
        _NC = build_nc()
    return _NC


def _make_in_maps(queries, keys, values, valid_lens, W_q, W_k, w_v):
    queries = np.ascontiguousarray(np.asarray(queries, dtype=np.float32))
    keys = np.ascontiguousarray(np.asarray(keys, dtype=np.float32))
    values = np.ascontiguousarray(np.asarray(values, dtype=np.float32))
    valid_lens = np.asarray(valid_lens)
    W_q = np.ascontiguousarray(np.asarray(W_q, dtype=np.float32))
    W_k = np.ascontiguousarray(np.asarray(W_k, dtype=np.float32))
    w_v = np.ascontiguousarray(np.asarray(w_v, dtype=np.float32).reshape(H, 1))
    ident = np.eye(128, dtype=np.float32)
    arange = np.arange(K)
    in_maps = []
    for b in range(B):
        vl = int(valid_lens[b])
        maskrow = np.where(arange >= vl, np.float32(-1.0e6), np.float32(0.0))
        maskbias = np.ascontiguousarray(
            np.broadcast_to(maskrow, (128, K)).astype(np.float32))
        in_maps.append({
            "queries": np.ascontiguousarray(queries[b]),
            "keys": np.ascontiguousarray(keys[b]),
            "values": np.ascontiguousarray(values[b]),
            "W_q": W_q, "W_k": W_k, "w_v": w_v,
            "maskbias": maskbias, "identity": ident,
        })
    return in_maps


def run_spmd(in_maps, **kwargs):
    nc = _get_nc()
    return run_bass_kernel_spmd(nc, in_maps, core_ids=list(range(N_CORES)), **kwargs)


def kernel(queries, keys, values, valid_lens, W_q, W_k, w_v):
    in_maps = _make_in_maps(queries, keys, values, valid_lens, W_q, W_k, w_v)
    res = run_spmd(in_maps)
    return np.stack([res.results[b]["out"] for b in range(B)]).astype(np.float32)


# revision 12
# speedup vs baseline: 1.2264x; 1.2264x over previous
"""Additive (Bahdanau) attention on Trainium2, data-parallel over batch on 8 NeuronCores.

Per core (one batch element):
  qT[h,q] = (queries @ W_q).T, kT[h,k] = (keys @ W_k).T        (PE, fp32; stored bf16)
  pre[h, (k,q)] = qT[h,q] + kT[h,k]                            (DVE tensor_scalar, bf16 4x)
  feat = tanh(pre)                                             (ACT, few big instructions)
  scores[q,k] = sum_h w_v[h]*feat[h,k,q]                       (PE: feat stationary, w_v moving,
                                                                column-accumulated in PSUM)
  scores += maskbias (-1e6 where k >= valid_len)               (DVE)
  e = exp(scores), sums = row-sums via ACT accum_out           (ACT)
  out[q,:] = (e.T @ values) / sums                             (PE transpose + PE matmul + DVE)
"""

import numpy as np

import concourse.bass as bass
import concourse.mybir as mybir
import concourse.tile as tile
from concourse import bacc
from concourse.bass_utils import run_bass_kernel_spmd

B, Q, K, H, D, DV = 8, 256, 256, 256, 256, 256
N_CORES = 8
F32 = mybir.dt.float32
BF16 = mybir.dt.bfloat16
AF = mybir.ActivationFunctionType
KC = 64            # keys per chunk (one big ACT instruction covers KC*Q elements)
NCHUNK = K // KC


def build_nc():
    nc = bacc.Bacc("TRN2", target_bir_lowering=False)
    d_q = nc.dram_tensor("queries", [Q, D], F32, kind="ExternalInput")
    d_k = nc.dram_tensor("keys", [K, D], F32, kind="ExternalInput")
    d_v = nc.dram_tensor("values", [K, DV], F32, kind="ExternalInput")
    d_wq = nc.dram_tensor("W_q", [D, H], F32, kind="ExternalInput")
    d_wk = nc.dram_tensor("W_k", [D, H], F32, kind="ExternalInput")
    d_wv = nc.dram_tensor("w_v", [H, 1], F32, kind="ExternalInput")
    d_mask = nc.dram_tensor("maskbias", [128, K], F32, kind="ExternalInput")
    d_id = nc.dram_tensor("identity", [128, 128], F32, kind="ExternalInput")
    d_out = nc.dram_tensor("out", [Q, DV], F32, kind="ExternalOutput")

    with tile.TileContext(nc) as tc:
        with (
            tc.tile_pool(name="sb", bufs=1) as sb,
            tc.tile_pool(name="feat", bufs=4) as feat_pool,
            tc.tile_pool(name="ps_scores", bufs=1, space=bass.MemorySpace.PSUM) as ps_s,
        ):
            # ------- persistent tiles -------
            ident = sb.tile([128, 128], F32, tag="ident")
            nc.sync.dma_start(ident[:], d_id[:])
            mask_sb = sb.tile([128, K], F32, tag="mask")
            nc.sync.dma_start(mask_sb[:], d_mask[:])
            wv_f = [sb.tile([128, 1], F32, tag=f"wvf{t}", name=f"wvf{t}") for t in range(2)]
            wv_b = [sb.tile([128, 1], BF16, tag=f"wvb{t}", name=f"wvb{t}") for t in range(2)]
            for t in range(2):
                nc.sync.dma_start(wv_f[t][:], d_wv[t * 128:(t + 1) * 128, :])
                nc.vector.tensor_copy(wv_b[t][:], wv_f[t][:])
            vals = [sb.tile([128, DV], F32, tag=f"vals{t}", name=f"vals{t}") for t in range(2)]
            for t in range(2):
                nc.sync.dma_start(vals[t][:], d_v[t * 128:(t + 1) * 128, :])
            qT = [sb.tile([128, Q], BF16, tag=f"qT{t}", name=f"qT{t}") for t in range(2)]
            kT = [sb.tile([128, K], BF16, tag=f"kT{t}", name=f"kT{t}") for t in range(2)]
            # kt2: each kT value duplicated into a bf16 pair -> enables TT 2x_1P packing
            kt2 = [sb.tile([128, 2 * K], BF16, tag=f"kt2{t}", name=f"kt2{t}") for t in range(2)]
            # one PSUM tile per (qt, t) so every matmul is an independent start/stop=True
            # (accumulation groups would block PE LDWEIGHTS pull-ahead and serialize MMs)
            s_ps = [[ps_s.tile([128, K], F32, tag=f"s{qt}_{t}", name=f"s{qt}_{t}")
                     for t in range(2)] for qt in range(2)]

            # ------- prep: load, transpose, project -------
            with (
                tc.tile_pool(name="prep_sb", bufs=1) as prep_sb,
                tc.tile_pool(name="ps_prep", bufs=2, space=bass.MemorySpace.PSUM) as ps_p,
            ):
                qraw = [prep_sb.tile([128, D], F32, tag=f"qraw{i}", name=f"qraw{i}") for i in range(2)]
                kraw = [prep_sb.tile([128, D], F32, tag=f"kraw{i}", name=f"kraw{i}") for i in range(2)]
                qTT = [prep_sb.tile([128, Q], F32, tag=f"qTT{j}", name=f"qTT{j}") for j in range(2)]
                kTT = [prep_sb.tile([128, K], F32, tag=f"kTT{j}", name=f"kTT{j}") for j in range(2)]
                wq_sb = [prep_sb.tile([128, H], F32, tag=f"wq{j}", name=f"wq{j}") for j in range(2)]
                wk_sb = [prep_sb.tile([128, H], F32, tag=f"wk{j}", name=f"wk{j}") for j in range(2)]
                for i in range(2):
                    nc.sync.dma_start(qraw[i][:], d_q[i * 128:(i + 1) * 128, :])
                    nc.sync.dma_start(kraw[i][:], d_k[i * 128:(i + 1) * 128, :])
                    nc.sync.dma_start(wq_sb[i][:], d_wq[i * 128:(i + 1) * 128, :])
                    nc.sync.dma_start(wk_sb[i][:], d_wk[i * 128:(i + 1) * 128, :])
                # transpose [x, d] -> [d, x] in 128x128 blocks via PE
                for raw, TT in ((qraw, qTT), (kraw, kTT)):
                    for i in range(2):
                        for j in range(2):
                            tp = ps_p.tile([128, 128], F32, tag="tp")
                            nc.tensor.transpose(tp[:], raw[i][:, j * 128:(j + 1) * 128], ident[:])
                            nc.vector.tensor_copy(TT[j][:, i * 128:(i + 1) * 128], tp[:])
                # projections (contract d): xT[t][h,x] = sum_d W[d, t*128+h] * xTT[d, x]
                for w_sb, TT, xT in ((wq_sb, qTT, qT), (wk_sb, kTT, kT)):
                    for t in range(2):
                        pj = ps_p.tile([128, 256], F32, tag="proj")
                        for j in range(2):
                            nc.tensor.matmul(pj[:], w_sb[j][:, t * 128:(t + 1) * 128], TT[j][:],
                                             start=(j == 0), stop=(j == 1))
                        nc.vector.tensor_copy(xT[t][:], pj[:])
                for t in range(2):
                    nc.vector.tensor_copy(
                        kt2[t][:].rearrange("p (k e) -> p k e", e=2),
                        kT[t][:].unsqueeze(2).broadcast_to((128, K, 2)))

            # ------- main loop: tanh features + w_v reduction -------
            for c in range(NCHUNK):
                k0 = c * KC
                feats = []
                for t in range(2):
                    feat = feat_pool.tile([128, KC * Q], BF16, tag="feat")
                    # pre[h, j, qp, e] = qT[h, 2qp+e] + kT[h, k0+j]; bf16 pair APs keep 2x mode
                    in0 = qT[t][:].rearrange("p (qp e) -> p qp e", e=2)
                    in0 = in0.unsqueeze(1).broadcast_to((128, KC, Q // 2, 2))
                    in1 = kt2[t][:, 2 * k0:2 * (k0 + KC)].rearrange("p (k e) -> p k e", e=2)
                    in1 = in1.unsqueeze(2).broadcast_to((128, KC, Q // 2, 2))
                    out = feat[:].rearrange("p (a b c) -> p a b c", a=KC, b=Q // 2)
                    nc.vector.tensor_add(out, in0, in1)
                    nc.scalar.activation(feat[:], feat[:], AF.Tanh)
                    feats.append(feat)
                for j in range(KC):
                    k = k0 + j
                    for qt in range(2):
                        for t in range(2):
                            nc.tensor.matmul(
                                s_ps[qt][t][:, k:k + 1],
                                feats[t][:, j * Q + qt * 128: j * Q + qt * 128 + 128],
                                wv_b[t][:],
                                start=True, stop=True)

            # ------- masked softmax + attention @ values -------
            exp_sb = [sb.tile([128, K], F32, tag=f"exp{qt}", name=f"exp{qt}") for qt in range(2)]
            expT = [sb.tile([128, Q], F32, tag=f"expT{kt}", name=f"expT{kt}") for kt in range(2)]
            sums = [sb.tile([128, 1], F32, tag=f"sum{qt}", name=f"sum{qt}") for qt in range(2)]
            recip = [sb.tile([128, 1], F32, tag=f"rcp{qt}", name=f"rcp{qt}") for qt in range(2)]
            out_sb = [sb.tile([128, DV], F32, tag=f"out{qt}", name=f"out{qt}") for qt in range(2)]
            with tc.tile_pool(name="ps_tail", bufs=2, space=bass.MemorySpace.PSUM) as ps_t:
                for qt in range(2):
                    # TT may read only one PSUM operand: stage s1+mask into SBUF first
                    nc.vector.tensor_add(exp_sb[qt][:], s_ps[qt][1][:], mask_sb[:])
                    nc.vector.tensor_add(s_ps[qt][0][:], s_ps[qt][0][:], exp_sb[qt][:])
                    nc.scalar.activation(exp_sb[qt][:], s_ps[qt][0][:], AF.Exp,
                                         accum_out=sums[qt][:])
                for qt in range(2):
                    for kt in range(2):
                        tx = ps_t.tile([128, 128], F32, tag="tx")
                        nc.tensor.transpose(tx[:], exp_sb[qt][:, kt * 128:(kt + 1) * 128], ident[:])
                        nc.vector.tensor_copy(expT[kt][:, qt * 128:(qt + 1) * 128], tx[:])
                for qt in range(2):
                    av = ps_t.tile([128, DV], F32, tag="av")
                    for kt in range(2):
                        nc.tensor.matmul(av[:], expT[kt][:, qt * 128:(qt + 1) * 128], vals[kt][:],
                                         start=(kt == 0), stop=(kt == 1))
                    nc.vector.reciprocal(recip[qt][:], sums[qt][:])
                    nc.vector.tensor_scalar_mul(out_sb[qt][:], av[:], recip[qt][:])
                    nc.sync.dma_start(d_out[qt * 128:(qt + 1) * 128, :], out_sb[qt][:])
    nc.compile()
    return nc


_NC = None


def _get_nc():
    global _NC
    if _NC is None:
        _NC = build_nc()
    return _NC


def _make_in_maps(queries, keys, values, valid_lens, W_q, W_k, w_v):
    queries = np.ascontiguousarray(np.asarray(queries, dtype=np.float32))
    keys = np.ascontiguousarray(np.asarray(keys, dtype=np.float32))
    values = np.ascontiguousarray(np.asarray(values, dtype=np.float32))
    valid_lens = np.asarray(valid_lens)
    W_q = np.ascontiguousarray(np.asarray(W_q, dtype=np.float32))
    W_k = np.ascontiguousarray(np.asarray(W_k, dtype=np.float32))
    w_v = np.ascontiguousarray(np.asarray(w_v, dtype=np.float32).reshape(H, 1))
    ident = np.eye(128, dtype=np.float32)
    arange = np.arange(K)
    in_maps = []
    for b in range(B):
        vl = int(valid_lens[b])
        maskrow = np.where(arange >= vl, np.float32(-1.0e6), np.float32(0.0))
        maskbias = np.ascontiguousarray(
            np.broadcast_to(maskrow, (128, K)).astype(np.float32))
        in_maps.append({
            "queries": np.ascontiguousarray(queries[b]),
            "keys": np.ascontiguousarray(keys[b]),
            "values": np.ascontiguousarray(values[b]),
            "W_q": W_q, "W_k": W_k, "w_v": w_v,
            "maskbias": maskbias, "identity": ident,
        })
    return in_maps


def run_spmd(in_maps, **kwargs):
    nc = _get_nc()
    return run_bass_kernel_spmd(nc, in_maps, core_ids=list(range(N_CORES)), **kwargs)


def kernel(queries, keys, values, valid_lens, W_q, W_k, w_v):
    in_maps = _make_in_maps(queries, keys, values, valid_lens, W_q, W_k, w_v)
    res = run_spmd(in_maps)
    return np.stack([res.results[b]["out"] for b in range(B)]).astype(np.float32)
